# revision 6
# baseline (speedup 1.0000x reference)
"""Trainium2 Bass kernel for nn_DuelingDQN (2-layer LSTM + dueling-advantage MLP).

Strategy
--------
Data-parallel over batch: B=4096 is split as 512 per NeuronCore across 8 cores;
weights are replicated. On each core everything is kept in a transposed layout
(features on the SBUF partition dim, batch on the free dim), so the sequential
T=100 recurrence runs as a chain of bf16 matmuls (fp32 PSUM accumulation):

    gates.T (1024 x 512) = W.T-slices.T @ [x_t.T ; h.T]

256-row feature tensors (h, c, per-gate activations) are stored "folded" as
(128, 2*512) SBUF tiles — free-dim halves are feature rows [0:128) / [128:256) —
which halves the elementwise-op count. The input x is pre-transposed and cast
to bf16 on the host (numpy), so the device never transposes anything.

Per step: 64 LSTM matmuls + 6 MLP matmuls on PE, gate sigmoid/tanh (+bias) on
ACT straight out of PSUM, cell updates on DVE (c stays fp32, h is written as
bf16 for the next matmul). The MLP head for step t is emitted in the middle of
step t+1 so PE never stalls waiting for h1's ACT/DVE tail.

The walrus build in this container encodes at most ONE sync-wait per
instruction; Tile emits several. `_split_multiwaits` post-processes the BIR
JSON, hoisting extra waits onto injected same-engine EventSemaphore
instructions immediately before the owner (engine streams are in-order, so
this is semantically identical).
"""

import json
import sys
import types
from contextlib import ExitStack

import numpy as np

sys.path.insert(0, "/opt/trn_rl_repo")

import ml_dtypes  # noqa: E402

N_CORES = 8
B, T, IN, H = 4096, 100, 140, 256
BC = B // N_CORES  # 512 batch per core
G = 4 * H  # 1024 gate rows
BF16 = ml_dtypes.bfloat16


# --------------------------------------------------------------------------
# BIR post-processing: split multi-wait instructions (see module docstring)
# --------------------------------------------------------------------------
def _split_multiwaits(bir: dict) -> int:
    ctr = 0
    for f in bir["functions"]:
        for blk in f["blocks"]:
            new_insts = []
            for ins in blk["instructions"]:
                si = ins.get("sync_info")
                waits = (si or {}).get("on_wait") or []
                if len(waits) > 1:
                    for w in waits[:-1]:
                        ctr += 1
                        new_insts.append(
                            {
                                "debug": ins.get("debug", 0),
                                "engine": ins["engine"],
                                "ins": [],
                                "outs": [],
                                "name": f"antsplitw-{ctr}",
                                "opcode": "EventSemaphore",
                                "sync_info": {"on_update": [], "on_wait": [w]},
                            }
                        )
                    si["on_wait"] = [waits[-1]]
                new_insts.append(ins)
            blk["instructions"] = new_insts
    return ctr


def _patch_bass(nc):
    import concourse.mybir as mybir

    def to_json_bytes(self):
        j = json.loads(mybir.module_to_json_bytes(self.m))
        _split_multiwaits(j)
        return json.dumps(j).encode()

    nc.to_json_bytes = types.MethodType(to_json_bytes, nc)
    return nc


# --------------------------------------------------------------------------
# Module build
# --------------------------------------------------------------------------
def build_module(b_a2_val: float, T_steps: int = T):
    import concourse.bass as bass
    import concourse.tile as tile
    from concourse import mybir

    f32 = mybir.dt.float32
    bf16 = mybir.dt.bfloat16
    AF = mybir.ActivationFunctionType
    ALU = mybir.AluOpType

    nc = bass.Bass("TRN2", target_bir_lowering=False, debug=False)

    xT_d = nc.dram_tensor("xT", (T_steps, IN, BC), bf16, kind="ExternalInput").ap()
    wih0_d = nc.dram_tensor("wih0", (IN, G), bf16, kind="ExternalInput").ap()
    whh0_d = nc.dram_tensor("whh0", (128, 2 * G), bf16, kind="ExternalInput").ap()
    wih1_d = nc.dram_tensor("wih1", (128, 2 * G), bf16, kind="ExternalInput").ap()
    whh1_d = nc.dram_tensor("whh1", (128, 2 * G), bf16, kind="ExternalInput").ap()
    wa1_d = nc.dram_tensor("wa1", (128, 2 * H), bf16, kind="ExternalInput").ap()
    wa2_d = nc.dram_tensor("wa2", (128, 2), bf16, kind="ExternalInput").ap()
    bias0_d = nc.dram_tensor("bias0", (128, 8), f32, kind="ExternalInput").ap()
    bias1_d = nc.dram_tensor("bias1", (128, 8), f32, kind="ExternalInput").ap()
    ba1_d = nc.dram_tensor("ba1", (128, 2), f32, kind="ExternalInput").ap()
    o_d = nc.dram_tensor("o", (T_steps, BC), f32, kind="ExternalOutput").ap()

    GATE_FUNCS = [AF.Sigmoid, AF.Sigmoid, AF.Tanh, AF.Sigmoid]  # i, f, g, o

    with tile.TileContext(nc) as tc, ExitStack() as ctx:
        persist = ctx.enter_context(tc.tile_pool(name="persist", bufs=1))
        xpool = ctx.enter_context(tc.tile_pool(name="xpool", bufs=4))
        gpool = ctx.enter_context(tc.tile_pool(name="gates_sb", bufs=2))
        tpool = ctx.enter_context(tc.tile_pool(name="tmp_sb", bufs=2))
        psg = ctx.enter_context(tc.tile_pool(name="ps_gates", bufs=3, space="PSUM"))
        pso = ctx.enter_context(tc.tile_pool(name="ps_out", bufs=2, space="PSUM"))

        def load(name, dram_ap, shape, dt):
            t = persist.tile(shape, dt, tag=name, name=name)
            nc.sync.dma_start(t[:], dram_ap)
            return t

        wih0a = load("wih0a", wih0_d[0:128, :], [128, G], bf16)
        wih0b = load("wih0b", wih0_d[128:IN, :], [IN - 128, G], bf16)
        whh0 = load("whh0", whh0_d[:], [128, 2 * G], bf16)
        wih1 = load("wih1", wih1_d[:], [128, 2 * G], bf16)
        whh1 = load("whh1", whh1_d[:], [128, 2 * G], bf16)
        wa1 = load("wa1", wa1_d[:], [128, 2 * H], bf16)
        wa2 = load("wa2", wa2_d[:], [128, 2], bf16)
        bias0 = load("bias0", bias0_d[:], [128, 8], f32)
        bias1 = load("bias1", bias1_d[:], [128, 8], f32)
        ba1 = load("ba1", ba1_d[:], [128, 2], f32)

        h0 = persist.tile([128, 2 * BC], bf16, tag="h0", name="h0")
        h1 = persist.tile([128, 2 * BC], bf16, tag="h1", name="h1")
        c0 = persist.tile([128, 2 * BC], f32, tag="c0", name="c0")
        c1 = persist.tile([128, 2 * BC], f32, tag="c1", name="c1")

        def lstm_layer(t, wih_k, whh, h, c, bias, lname, hh_first):
            """Emit one LSTM layer for step t.

            wih_k: list of (lhsT_tensor, col_base, rhs_ap) per input K-tile.
            whh:   folded (128, 2G) weight for the recurrent part.
            h, c:  folded state tiles (h read for t>0, both written).
            hh_first: put recurrent matmuls before input matmuls inside each
                      accumulation group (layer 1: h1[t-1] is ready before
                      h0[t]).
            """
            gates = []
            for g in range(4):
                ps = psg.tile([128, 2 * BC], f32, tag="gates", name=f"ps_{lname}{g}_{t}")
                sb = gpool.tile([128, 2 * BC], f32, tag=f"g{g}", name=f"sb_{lname}{g}_{t}")
                for j in range(2):
                    m = 2 * g + j
                    col = 128 * m
                    out = ps[:, j * BC : (j + 1) * BC]
                    ih_mms = [
                        (lhsT[:, cb + col : cb + col + 128], rhs)
                        for (lhsT, cb, rhs) in wih_k
                    ]
                    hh_mms = (
                        [
                            (whh[:, k * G + col : k * G + col + 128],
                             h[:, k * BC : (k + 1) * BC])
                            for k in range(2)
                        ]
                        if t > 0
                        else []
                    )
                    mms = hh_mms + ih_mms if hh_first else ih_mms + hh_mms
                    for idx, (lhsT, rhs) in enumerate(mms):
                        nc.tensor.matmul(out, lhsT, rhs,
                                         start=(idx == 0), stop=(idx == len(mms) - 1))
                    nc.scalar.activation(sb[:, j * BC : (j + 1) * BC], out,
                                         GATE_FUNCS[g], bias=bias[:, m : m + 1])
                gates.append(sb)
            gi, gf, gg, go = gates
            if t > 0:
                t1 = tpool.tile([128, 2 * BC], f32, tag="t1", name=f"t1_{lname}_{t}")
                nc.vector.tensor_mul(t1[:], gi[:], gg[:])
                nc.vector.tensor_mul(c[:], c[:], gf[:])
                nc.vector.tensor_add(c[:], c[:], t1[:])
            else:
                nc.vector.tensor_mul(c[:], gi[:], gg[:])
            tc_t = tpool.tile([128, 2 * BC], f32, tag="tanhc", name=f"tc_{lname}_{t}")
            nc.scalar.activation(tc_t[:], c[:], AF.Tanh)
            nc.vector.tensor_mul(h[:], go[:], tc_t[:])

        def mlp_head(t):
            """Advantage head for step t; reads current h1 contents."""
            ps_a = psg.tile([128, 2 * BC], f32, tag="gates", name=f"ps_a1_{t}")
            relu = tpool.tile([128, 2 * BC], bf16, tag="relu", name=f"relu_{t}")
            for j in range(2):
                out = ps_a[:, j * BC : (j + 1) * BC]
                for k in range(2):
                    nc.tensor.matmul(
                        out,
                        wa1[:, k * H + 128 * j : k * H + 128 * j + 128],
                        h1[:, k * BC : (k + 1) * BC],
                        start=(k == 0), stop=(k == 1),
                    )
                nc.vector.tensor_scalar(
                    relu[:, j * BC : (j + 1) * BC], out,
                    ba1[:, j : j + 1], 0.0, ALU.add, ALU.max,
                )
            ps_o = pso.tile([1, BC], f32, tag="po", name=f"ps_o_{t}")
            for k in range(2):
                nc.tensor.matmul(ps_o[:], wa2[:, k : k + 1],
                                 relu[:, k * BC : (k + 1) * BC],
                                 start=(k == 0), stop=(k == 1))
            osb = tpool.tile([1, BC], f32, tag="osb", name=f"osb_{t}")
            nc.vector.tensor_scalar(osb[:], ps_o[:], float(b_a2_val), None, ALU.add)
            nc.sync.dma_start(o_d[t : t + 1, :], osb[:])

        for t in range(T_steps):
            xa = xpool.tile([128, BC], bf16, tag="xa", name=f"xa_{t}")
            nc.sync.dma_start(xa[:], xT_d[t, 0:128, :])
            xb = xpool.tile([IN - 128, BC], bf16, tag="xb", name=f"xb_{t}")
            nc.sync.dma_start(xb[:], xT_d[t, 128:IN, :])

            lstm_layer(t, [(wih0a, 0, xa[:]), (wih0b, 0, xb[:])], whh0,
                       h0, c0, bias0, "l0", hh_first=False)
            if t > 0:
                mlp_head(t - 1)
            lstm_layer(t, [(wih1, 0, h0[:, 0:BC]), (wih1, G, h0[:, BC : 2 * BC])],
                       whh1, h1, c1, bias1, "l1", hh_first=True)
        mlp_head(T_steps - 1)

    return _patch_bass(nc)


# --------------------------------------------------------------------------
# Host-side input prep / output assembly
# --------------------------------------------------------------------------
def _fold(wT: np.ndarray) -> np.ndarray:
    """(2K, M) -> (128, 2M): free halves are K-rows [0:128) / [128:256)."""
    k2, m = wT.shape
    assert k2 == 256
    return np.ascontiguousarray(
        wT.reshape(2, 128, m).transpose(1, 0, 2).reshape(128, 2 * m)
    )


def prepare_in_maps(inputs: dict) -> list[dict]:
    f32 = np.float32
    W_ih0 = np.asarray(inputs["W_ih0"], f32)
    W_hh0 = np.asarray(inputs["W_hh0"], f32)
    W_ih1 = np.asarray(inputs["W_ih1"], f32)
    W_hh1 = np.asarray(inputs["W_hh1"], f32)
    W_a1 = np.asarray(inputs["W_a1"], f32)
    W_a2 = np.asarray(inputs["W_a2"], f32)

    shared = {
        "wih0": np.ascontiguousarray(W_ih0.T).astype(BF16),
        "whh0": _fold(W_hh0.T).astype(BF16),
        "wih1": _fold(W_ih1.T).astype(BF16),
        "whh1": _fold(W_hh1.T).astype(BF16),
        "wa1": _fold(W_a1.T).astype(BF16),
        "wa2": _fold(W_a2.T).astype(BF16),
        "bias0": np.ascontiguousarray(
            (np.asarray(inputs["b_ih0"], f32) + np.asarray(inputs["b_hh0"], f32))
            .reshape(8, 128).T),
        "bias1": np.ascontiguousarray(
            (np.asarray(inputs["b_ih1"], f32) + np.asarray(inputs["b_hh1"], f32))
            .reshape(8, 128).T),
        "ba1": np.ascontiguousarray(np.asarray(inputs["b_a1"], f32).reshape(2, 128).T),
    }

    x = np.asarray(inputs["x"], f32)  # (B, T, IN)
    xT = x.transpose(1, 2, 0)  # (T, IN, B) view
    in_maps = []
    for c in range(N_CORES):
        xc = xT[:, :, c * BC : (c + 1) * BC].astype(BF16)  # contiguous copy
        in_maps.append({"xT": xc, **shared})
    return in_maps


def assemble_output(results: list[dict]) -> np.ndarray:
    out_tb = np.concatenate([r["o"] for r in results], axis=1)  # (T, B)
    t_steps = out_tb.shape[0]
    return np.ascontiguousarray(out_tb.reshape(B, t_steps))


_module_cache: dict = {}


def get_module(b_a2_val: float):
    key = round(float(b_a2_val), 12)
    if key not in _module_cache:
        _module_cache[key] = build_module(float(b_a2_val))
    return _module_cache[key]


def kernel(**inputs) -> np.ndarray:
    from concourse import bass_utils

    b_a2_val = float(np.asarray(inputs["b_a2"], np.float32).reshape(-1)[0])
    nc = get_module(b_a2_val)
    in_maps = prepare_in_maps(inputs)
    res = bass_utils.run_bass_kernel_spmd(nc, in_maps, core_ids=list(range(N_CORES)))
    return assemble_output(res.results)


# revision 12
# speedup vs baseline: 4.2364x; 4.2364x over previous
"""Trainium2 Bass kernel for nn_DuelingDQN (2-layer LSTM + dueling-advantage MLP).

Strategy
--------
Data-parallel over batch: B=4096 is split as 512 per NeuronCore across 8 cores;
weights are replicated. On each core everything is kept in a transposed layout
(features on the SBUF partition dim, batch on the free dim), so the sequential
T=100 recurrence runs as a chain of bf16 matmuls (fp32 PSUM accumulation):

    gates.T (1024 x 512) = W.T-slices.T @ [x_t.T ; h.T]

256-row feature tensors (h, c, per-gate activations) are stored "folded" as
(128, 2*512) SBUF tiles — free-dim halves are feature rows [0:128) / [128:256) —
which halves the elementwise-op count. The input x is pre-transposed and cast
to bf16 on the host (numpy), so the device never transposes anything.

Per step: 64 LSTM matmuls + 6 MLP matmuls on PE, gate sigmoid/tanh (+bias) on
ACT straight out of PSUM, cell updates on DVE (c stays fp32, h is written as
bf16 for the next matmul). The MLP head for step t is emitted in the middle of
step t+1 so PE never stalls waiting for h1's ACT/DVE tail.

The walrus build in this container encodes at most ONE sync-wait per
instruction; Tile emits several. `_split_multiwaits` post-processes the BIR
JSON, hoisting extra waits onto injected same-engine EventSemaphore
instructions immediately before the owner (engine streams are in-order, so
this is semantically identical).
"""

import json
import sys
import types
from contextlib import ExitStack

import numpy as np

sys.path.insert(0, "/opt/trn_rl_repo")

import ml_dtypes  # noqa: E402

N_CORES = 8
B, T, IN, H = 4096, 100, 140, 256
BC = B // N_CORES  # 512 batch per core
G = 4 * H  # 1024 gate rows
BF16 = ml_dtypes.bfloat16


# --------------------------------------------------------------------------
# BIR post-processing: split multi-wait instructions (see module docstring)
# --------------------------------------------------------------------------
def _split_multiwaits(bir: dict) -> int:
    ctr = 0
    for f in bir["functions"]:
        for blk in f["blocks"]:
            new_insts = []
            for ins in blk["instructions"]:
                si = ins.get("sync_info")
                waits = (si or {}).get("on_wait") or []
                if len(waits) > 1:
                    for w in waits[:-1]:
                        ctr += 1
                        new_insts.append(
                            {
                                "debug": ins.get("debug", 0),
                                "engine": ins["engine"],
                                "ins": [],
                                "outs": [],
                                "name": f"antsplitw-{ctr}",
                                "opcode": "EventSemaphore",
                                "sync_info": {"on_update": [], "on_wait": [w]},
                            }
                        )
                    si["on_wait"] = [waits[-1]]
                new_insts.append(ins)
            blk["instructions"] = new_insts
    return ctr


def _patch_bass(nc):
    import concourse.mybir as mybir

    def to_json_bytes(self):
        j = json.loads(mybir.module_to_json_bytes(self.m))
        _split_multiwaits(j)
        return json.dumps(j).encode()

    nc.to_json_bytes = types.MethodType(to_json_bytes, nc)
    return nc


# --------------------------------------------------------------------------
# Module build
# --------------------------------------------------------------------------
def build_module(b_a2_val: float, T_steps: int = T):
    import concourse.bass as bass
    import concourse.tile as tile
    from concourse import mybir

    f32 = mybir.dt.float32
    bf16 = mybir.dt.bfloat16
    AF = mybir.ActivationFunctionType
    ALU = mybir.AluOpType

    nc = bass.Bass("TRN2", target_bir_lowering=False, debug=False)

    # x is extended with a constant ones-row (index IN) so the layer-0 bias
    # rides in the tail matmul (wih0 row IN = b0) — frees ACT from per-half
    # bias and lets layer-0 gates use single folded 1024-wide ACT ops.
    xT_d = nc.dram_tensor("xT", (T_steps, IN + 1, BC), bf16, kind="ExternalInput").ap()
    wih0_d = nc.dram_tensor("wih0", (IN + 1, G), bf16, kind="ExternalInput").ap()
    whh0_d = nc.dram_tensor("whh0", (128, 2 * G), bf16, kind="ExternalInput").ap()
    wih1_d = nc.dram_tensor("wih1", (128, 2 * G), bf16, kind="ExternalInput").ap()
    whh1_d = nc.dram_tensor("whh1", (128, 2 * G), bf16, kind="ExternalInput").ap()
    wa1_d = nc.dram_tensor("wa1", (128, 2 * H), bf16, kind="ExternalInput").ap()
    wa2_d = nc.dram_tensor("wa2", (128, 2), bf16, kind="ExternalInput").ap()
    bias1_d = nc.dram_tensor("bias1", (128, 8), f32, kind="ExternalInput").ap()
    ba1_d = nc.dram_tensor("ba1", (128, 2), f32, kind="ExternalInput").ap()
    o_d = nc.dram_tensor("o", (T_steps, BC), f32, kind="ExternalOutput").ap()

    GATE_FUNCS = [AF.Sigmoid, AF.Sigmoid, AF.Tanh, AF.Sigmoid]  # i, f, g, o

    with tile.TileContext(nc) as tc, ExitStack() as ctx:
        persist = ctx.enter_context(tc.tile_pool(name="persist", bufs=1))
        xpool = ctx.enter_context(tc.tile_pool(name="xpool", bufs=4))
        gpool = ctx.enter_context(tc.tile_pool(name="gates_sb", bufs=2))
        tpool = ctx.enter_context(tc.tile_pool(name="tmp_sb", bufs=2))
        psg = ctx.enter_context(tc.tile_pool(name="ps_gates", bufs=3, space="PSUM"))
        pso = ctx.enter_context(tc.tile_pool(name="ps_out", bufs=2, space="PSUM"))

        def load(name, dram_ap, shape, dt):
            t = persist.tile(shape, dt, tag=name, name=name)
            nc.sync.dma_start(t[:], dram_ap)
            return t

        wih0a = load("wih0a", wih0_d[0:128, :], [128, G], bf16)
        wih0b = load("wih0b", wih0_d[128 : IN + 1, :], [IN + 1 - 128, G], bf16)
        whh0 = load("whh0", whh0_d[:], [128, 2 * G], bf16)
        wih1 = load("wih1", wih1_d[:], [128, 2 * G], bf16)
        whh1 = load("whh1", whh1_d[:], [128, 2 * G], bf16)
        wa1 = load("wa1", wa1_d[:], [128, 2 * H], bf16)
        wa2 = load("wa2", wa2_d[:], [128, 2], bf16)
        bias1 = load("bias1", bias1_d[:], [128, 8], f32)
        ba1 = load("ba1", ba1_d[:], [128, 2], f32)

        h0 = persist.tile([128, 2 * BC], bf16, tag="h0", name="h0")
        h1 = persist.tile([128, 2 * BC], bf16, tag="h1", name="h1")
        c0 = persist.tile([128, 2 * BC], f32, tag="c0", name="c0")
        c1 = persist.tile([128, 2 * BC], f32, tag="c1", name="c1")

        def lstm_layer(t, wih_k, whh, h, c, bias, lname, hh_first):
            """Emit one LSTM layer for step t.

            wih_k: list of (lhsT_tensor, col_base, rhs_ap) per input K-tile.
            whh:   folded (128, 2G) weight for the recurrent part.
            h, c:  folded state tiles (h read for t>0, both written).
            hh_first: put recurrent matmuls before input matmuls inside each
                      accumulation group (layer 1: h1[t-1] is ready before
                      h0[t]).
            """
            gates = []
            for g in range(4):
                ps = psg.tile([128, 2 * BC], f32, tag="gates", name=f"ps_{lname}{g}_{t}")
                sb = gpool.tile([128, 2 * BC], bf16, tag=f"g{g}", name=f"sb_{lname}{g}_{t}")
                for j in range(2):
                    m = 2 * g + j
                    col = 128 * m
                    out = ps[:, j * BC : (j + 1) * BC]
                    ih_mms = [
                        (lhsT[:, cb + col : cb + col + 128], rhs)
                        for (lhsT, cb, rhs) in wih_k
                    ]
                    hh_mms = (
                        [
                            (whh[:, k * G + col : k * G + col + 128],
                             h[:, k * BC : (k + 1) * BC])
                            for k in range(2)
                        ]
                        if t > 0
                        else []
                    )
                    mms = hh_mms + ih_mms if hh_first else ih_mms + hh_mms
                    for idx, (lhsT, rhs) in enumerate(mms):
                        nc.tensor.matmul(out, lhsT, rhs,
                                         start=(idx == 0), stop=(idx == len(mms) - 1))
                    if bias is not None:
                        nc.scalar.activation(sb[:, j * BC : (j + 1) * BC], out,
                                             GATE_FUNCS[g], bias=bias[:, m : m + 1])
                if bias is None:
                    # bias already accumulated in PSUM via the ones-row matmul:
                    # one folded 1024-wide ACT op per gate
                    nc.scalar.activation(sb[:], ps[:], GATE_FUNCS[g])
                gates.append(sb)
            gi, gf, gg, go = gates
            if t > 0:
                t1 = tpool.tile([128, 2 * BC], bf16, tag="t1", name=f"t1_{lname}_{t}")
                nc.vector.tensor_mul(t1[:], gi[:], gg[:])
                nc.vector.tensor_mul(c[:], c[:], gf[:])
                nc.vector.tensor_add(c[:], c[:], t1[:])
            else:
                nc.vector.tensor_mul(c[:], gi[:], gg[:])
            tc_t = tpool.tile([128, 2 * BC], bf16, tag="tanhc", name=f"tc_{lname}_{t}")
            nc.scalar.activation(tc_t[:], c[:], AF.Tanh)
            nc.vector.tensor_mul(h[:], go[:], tc_t[:])

        def mlp_head(t):
            """Advantage head for step t; reads current h1 contents."""
            ps_a = psg.tile([128, 2 * BC], f32, tag="gates", name=f"ps_a1_{t}")
            relu = tpool.tile([128, 2 * BC], bf16, tag="relu", name=f"relu_{t}")
            for j in range(2):
                out = ps_a[:, j * BC : (j + 1) * BC]
                for k in range(2):
                    nc.tensor.matmul(
                        out,
                        wa1[:, k * H + 128 * j : k * H + 128 * j + 128],
                        h1[:, k * BC : (k + 1) * BC],
                        start=(k == 0), stop=(k == 1),
                    )
                nc.vector.tensor_scalar(
                    relu[:, j * BC : (j + 1) * BC], out,
                    ba1[:, j : j + 1], 0.0, ALU.add, ALU.max,
                )
            ps_o = pso.tile([1, BC], f32, tag="po", name=f"ps_o_{t}")
            for k in range(2):
                nc.tensor.matmul(ps_o[:], wa2[:, k : k + 1],
                                 relu[:, k * BC : (k + 1) * BC],
                                 start=(k == 0), stop=(k == 1))
            osb = tpool.tile([1, BC], f32, tag="osb", name=f"osb_{t}")
            nc.vector.tensor_scalar(osb[:], ps_o[:], float(b_a2_val), None, ALU.add)
            nc.sync.dma_start(o_d[t : t + 1, :], osb[:])

        for t in range(T_steps):
            xa = xpool.tile([128, BC], bf16, tag="xa", name=f"xa_{t}")
            nc.sync.dma_start(xa[:], xT_d[t, 0:128, :])
            xb = xpool.tile([IN + 1 - 128, BC], bf16, tag="xb", name=f"xb_{t}")
            nc.sync.dma_start(xb[:], xT_d[t, 128 : IN + 1, :])

            lstm_layer(t, [(wih0a, 0, xa[:]), (wih0b, 0, xb[:])], whh0,
                       h0, c0, None, "l0", hh_first=False)
            if t > 0:
                mlp_head(t - 1)
            lstm_layer(t, [(wih1, 0, h0[:, 0:BC]), (wih1, G, h0[:, BC : 2 * BC])],
                       whh1, h1, c1, bias1, "l1", hh_first=True)
        mlp_head(T_steps - 1)

    return _patch_bass(nc)


# --------------------------------------------------------------------------
# Host-side input prep / output assembly
# --------------------------------------------------------------------------
def _fold(wT: np.ndarray) -> np.ndarray:
    """(2K, M) -> (128, 2M): free halves are K-rows [0:128) / [128:256)."""
    k2, m = wT.shape
    assert k2 == 256
    return np.ascontiguousarray(
        wT.reshape(2, 128, m).transpose(1, 0, 2).reshape(128, 2 * m)
    )


def prepare_in_maps(inputs: dict) -> list[dict]:
    f32 = np.float32
    W_ih0 = np.asarray(inputs["W_ih0"], f32)
    W_hh0 = np.asarray(inputs["W_hh0"], f32)
    W_ih1 = np.asarray(inputs["W_ih1"], f32)
    W_hh1 = np.asarray(inputs["W_hh1"], f32)
    W_a1 = np.asarray(inputs["W_a1"], f32)
    W_a2 = np.asarray(inputs["W_a2"], f32)

    b0 = np.asarray(inputs["b_ih0"], f32) + np.asarray(inputs["b_hh0"], f32)
    # wih0 row IN carries b0 (multiplied by the ones-row appended to x)
    wih0_ext = np.concatenate([W_ih0.T, b0[None, :]], axis=0)  # (IN+1, G)

    shared = {
        "wih0": np.ascontiguousarray(wih0_ext).astype(BF16),
        "whh0": _fold(W_hh0.T).astype(BF16),
        "wih1": _fold(W_ih1.T).astype(BF16),
        "whh1": _fold(W_hh1.T).astype(BF16),
        "wa1": _fold(W_a1.T).astype(BF16),
        "wa2": _fold(W_a2.T).astype(BF16),
        "bias1": np.ascontiguousarray(
            (np.asarray(inputs["b_ih1"], f32) + np.asarray(inputs["b_hh1"], f32))
            .reshape(8, 128).T),
        "ba1": np.ascontiguousarray(np.asarray(inputs["b_a1"], f32).reshape(2, 128).T),
    }

    x = np.asarray(inputs["x"], f32)  # (B, T, IN)
    t_steps = x.shape[1]
    xT = x.transpose(1, 2, 0)  # (T, IN, B) view
    in_maps = []
    for c in range(N_CORES):
        xc = np.empty((t_steps, IN + 1, BC), BF16)
        xc[:, :IN, :] = xT[:, :, c * BC : (c + 1) * BC].astype(BF16)
        xc[:, IN, :] = np.ones((), BF16)
        in_maps.append({"xT": xc, **shared})
    return in_maps


def assemble_output(results: list[dict]) -> np.ndarray:
    out_tb = np.concatenate([r["o"] for r in results], axis=1)  # (T, B)
    t_steps = out_tb.shape[0]
    return np.ascontiguousarray(out_tb.reshape(B, t_steps))


_module_cache: dict = {}


def get_module(b_a2_val: float):
    key = round(float(b_a2_val), 12)
    if key not in _module_cache:
        _module_cache[key] = build_module(float(b_a2_val))
    return _module_cache[key]


def kernel(**inputs) -> np.ndarray:
    from concourse import bass_utils

    b_a2_val = float(np.asarray(inputs["b_a2"], np.float32).reshape(-1)[0])
    nc = get_module(b_a2_val)
    in_maps = prepare_in_maps(inputs)
    res = bass_utils.run_bass_kernel_spmd(nc, in_maps, core_ids=list(range(N_CORES)))
    return assemble_output(res.results)


# revision 14
# speedup vs baseline: 6.5473x; 1.5455x over previous
"""Trainium2 Bass kernel for nn_DuelingDQN (2-layer LSTM + dueling-advantage MLP).

Strategy
--------
Data-parallel over batch: B=4096 is split as 512 per NeuronCore across 8 cores;
weights are replicated. On each core everything is kept in a transposed layout
(features on the SBUF partition dim, batch on the free dim), so the sequential
T=100 recurrence runs as a chain of bf16 matmuls (fp32 PSUM accumulation):

    gates.T (1024 x 512) = W.T-slices.T @ [x_t.T ; h.T]

256-row feature tensors (h, c, per-gate activations) are stored "folded" as
(128, 2*512) SBUF tiles — free-dim halves are feature rows [0:128) / [128:256) —
which halves the elementwise-op count. The input x is pre-transposed and cast
to bf16 on the host (numpy), so the device never transposes anything.

Per step: 64 LSTM matmuls + 6 MLP matmuls on PE, gate sigmoid/tanh (+bias) on
ACT straight out of PSUM, cell updates on DVE (c stays fp32, h is written as
bf16 for the next matmul). The MLP head for step t is emitted in the middle of
step t+1 so PE never stalls waiting for h1's ACT/DVE tail.

The walrus build in this container encodes at most ONE sync-wait per
instruction; Tile emits several. `_split_multiwaits` post-processes the BIR
JSON, hoisting extra waits onto injected same-engine EventSemaphore
instructions immediately before the owner (engine streams are in-order, so
this is semantically identical).
"""

import json
import sys
import types
from contextlib import ExitStack

import numpy as np

sys.path.insert(0, "/opt/trn_rl_repo")

import ml_dtypes  # noqa: E402

N_CORES = 8
B, T, IN, H = 4096, 100, 140, 256
BC = B // N_CORES  # 512 batch per core
G = 4 * H  # 1024 gate rows
BF16 = ml_dtypes.bfloat16


# --------------------------------------------------------------------------
# BIR post-processing: split multi-wait instructions (see module docstring)
# --------------------------------------------------------------------------
def _split_multiwaits(bir: dict) -> int:
    ctr = 0
    for f in bir["functions"]:
        for blk in f["blocks"]:
            new_insts = []
            for ins in blk["instructions"]:
                si = ins.get("sync_info")
                waits = (si or {}).get("on_wait") or []
                if len(waits) > 1:
                    for w in waits[:-1]:
                        ctr += 1
                        new_insts.append(
                            {
                                "debug": ins.get("debug", 0),
                                "engine": ins["engine"],
                                "ins": [],
                                "outs": [],
                                "name": f"antsplitw-{ctr}",
                                "opcode": "EventSemaphore",
                                "sync_info": {"on_update": [], "on_wait": [w]},
                            }
                        )
                    si["on_wait"] = [waits[-1]]
                new_insts.append(ins)
            blk["instructions"] = new_insts
    return ctr


def _patch_bass(nc):
    import concourse.mybir as mybir

    def to_json_bytes(self):
        j = json.loads(mybir.module_to_json_bytes(self.m))
        _split_multiwaits(j)
        return json.dumps(j).encode()

    nc.to_json_bytes = types.MethodType(to_json_bytes, nc)
    return nc


# --------------------------------------------------------------------------
# Module build
# --------------------------------------------------------------------------
def build_module(b_a2_val: float, T_steps: int = T):
    import concourse.bass as bass
    import concourse.tile as tile
    from concourse import mybir

    f32 = mybir.dt.float32
    bf16 = mybir.dt.bfloat16
    AF = mybir.ActivationFunctionType
    ALU = mybir.AluOpType

    nc = bass.Bass("TRN2", target_bir_lowering=False, debug=False)

    # x is extended with a constant ones-row (index IN) so the layer-0 bias
    # rides in the tail matmul (wih0 row IN = b0) — frees ACT from per-half
    # bias and lets layer-0 gates use single folded 1024-wide ACT ops.
    xT_d = nc.dram_tensor("xT", (T_steps, IN + 1, BC), bf16, kind="ExternalInput").ap()
    wih0_d = nc.dram_tensor("wih0", (IN + 1, G), bf16, kind="ExternalInput").ap()
    whh0_d = nc.dram_tensor("whh0", (128, 2 * G), bf16, kind="ExternalInput").ap()
    wih1_d = nc.dram_tensor("wih1", (128, 2 * G), bf16, kind="ExternalInput").ap()
    whh1_d = nc.dram_tensor("whh1", (128, 2 * G), bf16, kind="ExternalInput").ap()
    wa1_d = nc.dram_tensor("wa1", (128, 2 * H), bf16, kind="ExternalInput").ap()
    wa2_d = nc.dram_tensor("wa2", (128, 2), bf16, kind="ExternalInput").ap()
    bias1_d = nc.dram_tensor("bias1", (128, 8), f32, kind="ExternalInput").ap()
    ba1_d = nc.dram_tensor("ba1", (128, 2), f32, kind="ExternalInput").ap()
    o_d = nc.dram_tensor("o", (T_steps, BC), f32, kind="ExternalOutput").ap()

    GATE_FUNCS = [AF.Sigmoid, AF.Sigmoid, AF.Tanh, AF.Sigmoid]  # i, f, g, o

    with tile.TileContext(nc) as tc, ExitStack() as ctx:
        persist = ctx.enter_context(tc.tile_pool(name="persist", bufs=1))
        xpool = ctx.enter_context(tc.tile_pool(name="xpool", bufs=4))
        gpool = ctx.enter_context(tc.tile_pool(name="gates_sb", bufs=2))
        tpool = ctx.enter_context(tc.tile_pool(name="tmp_sb", bufs=2))
        psg = ctx.enter_context(tc.tile_pool(name="ps_gates", bufs=3, space="PSUM"))
        pso = ctx.enter_context(tc.tile_pool(name="ps_out", bufs=2, space="PSUM"))

        def load(name, dram_ap, shape, dt):
            t = persist.tile(shape, dt, tag=name, name=name)
            nc.sync.dma_start(t[:], dram_ap)
            return t

        wih0a = load("wih0a", wih0_d[0:128, :], [128, G], bf16)
        wih0b = load("wih0b", wih0_d[128 : IN + 1, :], [IN + 1 - 128, G], bf16)
        whh0 = load("whh0", whh0_d[:], [128, 2 * G], bf16)
        wih1 = load("wih1", wih1_d[:], [128, 2 * G], bf16)
        whh1 = load("whh1", whh1_d[:], [128, 2 * G], bf16)
        wa1 = load("wa1", wa1_d[:], [128, 2 * H], bf16)
        wa2 = load("wa2", wa2_d[:], [128, 2], bf16)
        bias1 = load("bias1", bias1_d[:], [128, 8], f32)
        ba1 = load("ba1", ba1_d[:], [128, 2], f32)

        h0 = persist.tile([128, 2 * BC], bf16, tag="h0", name="h0")
        h1 = persist.tile([128, 2 * BC], bf16, tag="h1", name="h1")
        c0 = persist.tile([128, 2 * BC], f32, tag="c0", name="c0")
        c1 = persist.tile([128, 2 * BC], f32, tag="c1", name="c1")

        def lstm_layer(t, wih_k, whh, h, c, bias, lname, hh_first):
            """Emit one LSTM layer for step t.

            wih_k: list of (lhsT_tensor, col_base, rhs_ap) per input K-tile.
            whh:   folded (128, 2G) weight for the recurrent part.
            h, c:  folded state tiles (h read for t>0, both written).
            hh_first: put recurrent matmuls before input matmuls inside each
                      accumulation group (layer 1: h1[t-1] is ready before
                      h0[t]).
            """
            def make_tiles(g):
                ps = psg.tile([128, 2 * BC], f32, tag="gates", name=f"ps_{lname}{g}_{t}")
                sb = gpool.tile([128, 2 * BC], bf16, tag=f"g{g}",
                                name=f"sb_{lname}{g}_{t}")
                return ps, sb

            def mm_lists(g, j):
                col = 128 * (2 * g + j)
                ih = [(lhsT[:, cb + col : cb + col + 128], rhs)
                      for (lhsT, cb, rhs) in wih_k]
                hh = ([(whh[:, k * G + col : k * G + col + 128],
                        h[:, k * BC : (k + 1) * BC]) for k in range(2)]
                      if t > 0 else [])
                return ih, hh

            def emit_mms(out, mms, start, stop):
                for idx, (lhsT, rhs) in enumerate(mms):
                    nc.tensor.matmul(out, lhsT, rhs,
                                     start=start and idx == 0,
                                     stop=stop and idx == len(mms) - 1)

            def act_gate(g, ps, sb):
                if bias is not None:
                    for j in range(2):
                        m = 2 * g + j
                        nc.scalar.activation(sb[:, j * BC : (j + 1) * BC],
                                             ps[:, j * BC : (j + 1) * BC],
                                             GATE_FUNCS[g], bias=bias[:, m : m + 1])
                else:
                    # bias already accumulated in PSUM via the ones-row matmul:
                    # one folded 1024-wide ACT op per gate
                    nc.scalar.activation(sb[:], ps[:], GATE_FUNCS[g])

            gates = [None] * 4
            if hh_first and t > 0:
                # pairs of gates: all recurrent (hh) matmuls first, then input
                # (ih) matmuls — covers the h0[t] ACT/DVE tail with PE work
                for g0 in (0, 2):
                    tiles = {g: make_tiles(g) for g in (g0, g0 + 1)}
                    for g in (g0, g0 + 1):
                        ps, sb = tiles[g]
                        gates[g] = sb
                        for j in range(2):
                            _, hh = mm_lists(g, j)
                            emit_mms(ps[:, j * BC : (j + 1) * BC], hh,
                                     start=True, stop=False)
                    for g in (g0, g0 + 1):
                        ps, sb = tiles[g]
                        for j in range(2):
                            ih, _ = mm_lists(g, j)
                            emit_mms(ps[:, j * BC : (j + 1) * BC], ih,
                                     start=False, stop=True)
                        act_gate(g, ps, sb)
            else:
                for g in range(4):
                    ps, sb = make_tiles(g)
                    gates[g] = sb
                    for j in range(2):
                        ih, hh = mm_lists(g, j)
                        mms = hh + ih if (hh_first and t > 0) else ih + hh
                        emit_mms(ps[:, j * BC : (j + 1) * BC], mms,
                                 start=True, stop=True)
                    act_gate(g, ps, sb)
            gi, gf, gg, go = gates
            if t > 0:
                t1 = tpool.tile([128, 2 * BC], bf16, tag="t1", name=f"t1_{lname}_{t}")
                nc.vector.tensor_mul(t1[:], gi[:], gg[:])
                nc.vector.tensor_mul(c[:], c[:], gf[:])
                nc.vector.tensor_add(c[:], c[:], t1[:])
            else:
                nc.vector.tensor_mul(c[:], gi[:], gg[:])
            tc_t = tpool.tile([128, 2 * BC], bf16, tag="tanhc", name=f"tc_{lname}_{t}")
            nc.scalar.activation(tc_t[:], c[:], AF.Tanh)
            nc.vector.tensor_mul(h[:], go[:], tc_t[:])

        def mlp_head(t):
            """Advantage head for step t; reads current h1 contents."""
            ps_a = psg.tile([128, 2 * BC], f32, tag="gates", name=f"ps_a1_{t}")
            relu = tpool.tile([128, 2 * BC], bf16, tag="relu", name=f"relu_{t}")
            for j in range(2):
                out = ps_a[:, j * BC : (j + 1) * BC]
                for k in range(2):
                    nc.tensor.matmul(
                        out,
                        wa1[:, k * H + 128 * j : k * H + 128 * j + 128],
                        h1[:, k * BC : (k + 1) * BC],
                        start=(k == 0), stop=(k == 1),
                    )
                nc.scalar.activation(relu[:, j * BC : (j + 1) * BC], out,
                                     AF.Relu, bias=ba1[:, j : j + 1])
            ps_o = pso.tile([1, BC], f32, tag="po", name=f"ps_o_{t}")
            for k in range(2):
                nc.tensor.matmul(ps_o[:], wa2[:, k : k + 1],
                                 relu[:, k * BC : (k + 1) * BC],
                                 start=(k == 0), stop=(k == 1))
            osb = tpool.tile([1, BC], f32, tag="osb", name=f"osb_{t}")
            nc.vector.tensor_scalar(osb[:], ps_o[:], float(b_a2_val), None, ALU.add)
            nc.sync.dma_start(o_d[t : t + 1, :], osb[:])

        for t in range(T_steps):
            xa = xpool.tile([128, BC], bf16, tag="xa", name=f"xa_{t}")
            nc.sync.dma_start(xa[:], xT_d[t, 0:128, :])
            xb = xpool.tile([IN + 1 - 128, BC], bf16, tag="xb", name=f"xb_{t}")
            nc.sync.dma_start(xb[:], xT_d[t, 128 : IN + 1, :])

            lstm_layer(t, [(wih0a, 0, xa[:]), (wih0b, 0, xb[:])], whh0,
                       h0, c0, None, "l0", hh_first=False)
            if t > 0:
                mlp_head(t - 1)
            lstm_layer(t, [(wih1, 0, h0[:, 0:BC]), (wih1, G, h0[:, BC : 2 * BC])],
                       whh1, h1, c1, bias1, "l1", hh_first=True)
        mlp_head(T_steps - 1)

    return _patch_bass(nc)


# --------------------------------------------------------------------------
# Host-side input prep / output assembly
# --------------------------------------------------------------------------
def _fold(wT: np.ndarray) -> np.ndarray:
    """(2K, M) -> (128, 2M): free halves are K-rows [0:128) / [128:256)."""
    k2, m = wT.shape
    assert k2 == 256
    return np.ascontiguousarray(
        wT.reshape(2, 128, m).transpose(1, 0, 2).reshape(128, 2 * m)
    )


def prepare_in_maps(inputs: dict) -> list[dict]:
    f32 = np.float32
    W_ih0 = np.asarray(inputs["W_ih0"], f32)
    W_hh0 = np.asarray(inputs["W_hh0"], f32)
    W_ih1 = np.asarray(inputs["W_ih1"], f32)
    W_hh1 = np.asarray(inputs["W_hh1"], f32)
    W_a1 = np.asarray(inputs["W_a1"], f32)
    W_a2 = np.asarray(inputs["W_a2"], f32)

    b0 = np.asarray(inputs["b_ih0"], f32) + np.asarray(inputs["b_hh0"], f32)
    # wih0 row IN carries b0 (multiplied by the ones-row appended to x)
    wih0_ext = np.concatenate([W_ih0.T, b0[None, :]], axis=0)  # (IN+1, G)

    shared = {
        "wih0": np.ascontiguousarray(wih0_ext).astype(BF16),
        "whh0": _fold(W_hh0.T).astype(BF16),
        "wih1": _fold(W_ih1.T).astype(BF16),
        "whh1": _fold(W_hh1.T).astype(BF16),
        "wa1": _fold(W_a1.T).astype(BF16),
        "wa2": _fold(W_a2.T).astype(BF16),
        "bias1": np.ascontiguousarray(
            (np.asarray(inputs["b_ih1"], f32) + np.asarray(inputs["b_hh1"], f32))
            .reshape(8, 128).T),
        "ba1": np.ascontiguousarray(np.asarray(inputs["b_a1"], f32).reshape(2, 128).T),
    }

    x = np.asarray(inputs["x"], f32)  # (B, T, IN)
    t_steps = x.shape[1]
    xT = x.transpose(1, 2, 0)  # (T, IN, B) view
    in_maps = []
    for c in range(N_CORES):
        xc = np.empty((t_steps, IN + 1, BC), BF16)
        xc[:, :IN, :] = xT[:, :, c * BC : (c + 1) * BC].astype(BF16)
        xc[:, IN, :] = np.ones((), BF16)
        in_maps.append({"xT": xc, **shared})
    return in_maps


def assemble_output(results: list[dict]) -> np.ndarray:
    out_tb = np.concatenate([r["o"] for r in results], axis=1)  # (T, B)
    t_steps = out_tb.shape[0]
    return np.ascontiguousarray(out_tb.reshape(B, t_steps))


_module_cache: dict = {}


def get_module(b_a2_val: float):
    key = round(float(b_a2_val), 12)
    if key not in _module_cache:
        _module_cache[key] = build_module(float(b_a2_val))
    return _module_cache[key]


def kernel(**inputs) -> np.ndarray:
    from concourse import bass_utils

    b_a2_val = float(np.asarray(inputs["b_a2"], np.float32).reshape(-1)[0])
    nc = get_module(b_a2_val)
    in_maps = prepare_in_maps(inputs)
    res = bass_utils.run_bass_kernel_spmd(nc, in_maps, core_ids=list(range(N_CORES)))
    return assemble_output(res.results)
